# revision 21
# baseline (speedup 1.0000x reference)
"""Trainium2 Bass kernel for CLSProcess: diagonal linear recurrence
state_t = y_t * state_{t-1} + x_t * z_t over [B=8, T=4096, units=1024].

Sharding: batch across the 8 cores (one batch element per core).

v4 design:
  - the ONLY input DMAs are full-row gpsimd SWDGE cast-DMAs (f32->bf16
    in flight), one per 4-block group with a "(a b) c -> b a c"
    rearrange (4104-byte descriptors).  4-byte column gathers are
    catastrophic (the DMA queue burns ~7-16ns per tiny descriptor
    serially, ~55us for a [4096,1] column) and are completely avoided.
  - x / y0 columns per block come from one strided [128,4,2] engine
    copy per group (bf16 -> f32); the y ROW needed by the M-scan comes
    from a per-block PE transpose ([128,1] -> [1,128] bf16) + one Act
    row copy into a persistent [1, 32, 128] f32 row buffer whose
    block-start positions stay 0 (scan reset).
  - all matmuls bf16 (f32r lowers to 4-pass fp32 on this toolchain).
    lhsT = msc[s,t] = x_s*M[t,s] via a DVE tensor_scalar; M built by a
    DVE tensor_tensor_scan over the identity, 4 blocks per scan.
  - carry: po[t,:] += p_t*prev[127,:] as a K=128 rank-1 bf16 matmul.
    sel tiles are 10 rotating statics: rows 0:64 zeroed once at
    prologue, rows 64:128 rewritten per block (gpsimd broadcast of
    p_t + masked Act activation) -- halves the broadcast cost.
  - software pipeline: scans 2 groups ahead, per-block prep 8 blocks
    ahead, carries 2 blocks behind mains (PE never waits on drains),
    output bf16 (host upconverts), 2 blocks per output DMA on the SP
    queue.
"""

import numpy as np

import concourse.bacc as bacc
import concourse.bass as bass
import concourse.mybir as mybir
import concourse.tile as tile
from concourse.bass_utils import run_bass_kernel_spmd

B = 8
T = 4096
F = 1026
U = 1024
L = 128
G = 4  # blocks per group (one scan / one cast-DMA per group)
OB = 2  # blocks per output DMA
NSEL = 12
f32 = mybir.dt.float32
f32r = mybir.dt.float32r
bf16 = mybir.dt.bfloat16
Copy = mybir.ActivationFunctionType.Copy


def build_nc(t_total: int = T) -> bass.Bass:
    nb = t_total // L
    ng = nb // G
    nc = bacc.Bacc()
    inp = nc.dram_tensor("inp", [t_total, F], f32, kind="ExternalInput")
    out = nc.dram_tensor("out", [t_total, U], bf16, kind="ExternalOutput")
    ident4_d = nc.inline_tensor(
        np.tile(np.eye(L, dtype=np.float32), (1, G)), name="ident4"
    )
    e127c_np = np.zeros((L, 1), dtype=np.float32)
    e127c_np[L - 1, 0] = 1.0
    e127c_d = nc.inline_tensor(e127c_np, name="e127c")

    with tile.TileContext(nc) as tc:
        with (
            tc.tile_pool(name="const", bufs=1) as constp,
            tc.tile_pool(name="yrow", bufs=1) as yrowp,
            tc.tile_pool(name="zpool", bufs=8) as zpool,
            tc.tile_pool(name="xyfpool", bufs=5) as xyfpool,
            tc.tile_pool(name="mpool", bufs=3) as mpool,
            tc.tile_pool(name="mscpool", bufs=14) as mscpool,
            tc.tile_pool(name="rowpool", bufs=14) as rowpool,
            tc.tile_pool(name="bcpool", bufs=4) as bcpool,
            tc.tile_pool(name="pbpool", bufs=14) as pbpool,
            tc.tile_pool(name="selpool", bufs=14) as selpool,
            tc.tile_pool(name="otbpool", bufs=3) as otbpool,
            tc.tile_pool(name="ps_out", bufs=3, space="PSUM") as ps_out_pool,
            tc.tile_pool(name="ps_small", bufs=2, space="PSUM") as ps_small_pool,
        ):
            ident4 = constp.tile([L, G * L], f32, tag="ident4")
            nc.sync.dma_start(ident4[:], ident4_d[:, :])
            e127c = constp.tile([L, 1], f32, tag="e127c")
            nc.sync.dma_start(e127c[:], e127c_d[:, :])
            # bf16 identity for the bf16 y-column transposes (gpsimd
            # cast-DMA: engines cannot mix 16/32-bit matmul operands)
            identb = constp.tile([L, L], bf16, tag="identb")
            nc.gpsimd.dma_start(identb[:], ident4_d[:, 0:L])

            # persistent y rows, [1, block, step]; block-start positions
            # stay 0 so the M-scan resets at each block boundary
            yrow3 = yrowp.tile([1, nb, L], f32, tag="yrow3")
            nc.vector.memset(yrow3[0:1, :, 0:1], 0.0)

            zts = {}
            xyfs = {}
            ybcs = {}
            mt4s = {}
            mscs = {}
            sels = {}
            pos = {}
            otbs = {}

            def dispatch_z(g: int):
                r0 = g * G * L
                zt = zpool.tile([L, G * F], bf16, tag="zt")
                nc.gpsimd.dma_start(
                    zt[:],
                    inp[r0 : r0 + G * L, 0:F].rearrange("(a b) c -> b a c", a=G),
                )
                zts[g] = zt

            def prep_y(g: int):
                # x/y0 sideband columns (f32) + y rows + broadcast for group g
                zt = zts[g]
                z3 = zt[:].rearrange("b (a c) -> b a c", a=G)
                xyf = xyfpool.tile([L, G * 2], f32, tag="xyf")
                nc.vector.tensor_copy(xyf[:], z3[:, :, 0:2])
                xyfs[g] = xyf
                for j in range(G):
                    k = g * G + j
                    psm = ps_small_pool.tile([1, L], bf16, tag="psm")
                    nc.tensor.transpose(psm[0:1, :], zt[:, j * F + 1 : j * F + 2], identb[:])
                    nc.scalar.copy(yrow3[0:1, k : k + 1, 1:L], psm[0:1, 1:L])
                ybc = bcpool.tile([L, G * L], f32, tag="ybc")
                nc.gpsimd.partition_broadcast(
                    ybc[:], yrow3[0:1, G * g : G * (g + 1), :]
                )
                ybcs[g] = ybc

            def scan_group(g: int):
                mt4 = mpool.tile([L, G * L], f32r, tag="mt4")
                nc.vector.tensor_tensor_scan(
                    mt4[:],
                    ybcs.pop(g)[:],
                    ident4[:],
                    0.0,
                    mybir.AluOpType.mult,
                    mybir.AluOpType.add,
                )
                mt4s[g] = mt4

            def prep_block(k: int):
                # weights + carry selector for block k (runs well ahead of PE)
                g, j = divmod(k, G)
                mt4 = mt4s[g]
                mtk = mt4[:, L * j : L * j + L]
                xyf = xyfs[g]
                # bf16 weights: msc[s, t] = x_s * M[t, s]
                msc = mscpool.tile([L, L], bf16, tag="msc")
                nc.vector.tensor_scalar_mul(msc[:], mtk, xyf[:, 2 * j : 2 * j + 1])
                mscs[k] = msc
                if k > 0:
                    # p_t = prod_{r=block_start..t} y_r = y_0 * mt[0, t]
                    y0 = xyf[0:1, 2 * j + 1 : 2 * j + 2]
                    prow = rowpool.tile([1, L], f32, tag="prow")
                    nc.vector.tensor_scalar_mul(prow[:], mtk[0:1, :], y0)
                    # sel[s, t] = I[s==127] * p_t
                    pb = pbpool.tile([L, L], f32, tag="pb")
                    nc.gpsimd.partition_broadcast(pb[:], prow[0:1, :])
                    sel = selpool.tile([L, L], bf16, tag="sel")
                    nc.scalar.activation(sel[:], pb[:], Copy, scale=e127c[:])
                    sels[k] = sel

            def mains(k: int):
                g, j = divmod(k, G)
                po = ps_out_pool.tile([L, U], f32, tag="po")
                msc = mscs.pop(k)
                zt = zts[g]
                for jj in (0, 512):
                    nc.tensor.matmul(
                        po[:, jj : jj + 512],
                        msc[:],
                        zt[:, j * F + 2 + jj : j * F + 2 + jj + 512],
                        start=True,
                        stop=(k == 0),
                    )
                pos[k] = po

            def back(k: int):
                # carry accumulation + drain + output for block k
                po = pos.pop(k)
                if k > 0:
                    # po[t, :] += p_t * prev[127, :]
                    sel = sels.pop(k)
                    pt, pc = otbs[k - 1]
                    for jj in (0, 512):
                        nc.tensor.matmul(
                            po[:, jj : jj + 512],
                            sel[:],
                            pt[:, pc + jj : pc + jj + 512],
                            start=False,
                            stop=True,
                        )
                    otbs.pop(k - 1, None)
                # single bf16 drain, split DVE/Act; OB blocks share one
                # otb tile -> one batched output DMA
                h = k % OB
                if h == 0:
                    otb = otbpool.tile([L, OB * U], bf16, tag="otb")
                    otbs["cur"] = otb
                otb = otbs["cur"]
                c0 = h * U
                nc.vector.tensor_copy(otb[:, c0 : c0 + 384], po[:, 0:384])
                nc.scalar.copy(otb[:, c0 + 384 : c0 + U], po[:, 384:U])
                otbs[k] = (otb, c0)
                if h == OB - 1:
                    r0b = (k - OB + 1) * L
                    nc.sync.dma_start(
                        out[r0b : r0b + OB * L, :].rearrange(
                            "(a b) c -> b a c", a=OB
                        ),
                        otb[:],
                    )

            # prologue: all z cast-DMAs dispatched back-to-back on the
            # SWDGE queue; y/scan/weights prep pipelined ahead
            for g in range(ng):
                dispatch_z(g)
            prep_y(0)
            prep_y(1)
            prep_y(2)
            scan_group(0)
            scan_group(1)
            PREP_AHEAD = 8
            for kk in range(PREP_AHEAD):
                prep_block(kk)
            for k in range(nb):
                g = k // G
                if k % G == 0:
                    if g + 3 <= ng - 1:
                        prep_y(g + 3)
                    if g + 2 <= ng - 1:
                        scan_group(g + 2)
                if k + PREP_AHEAD < nb:
                    prep_block(k + PREP_AHEAD)
                mains(k)
                if k >= 2:
                    back(k - 2)
            back(nb - 2)
            back(nb - 1)
    nc.finalize()
    return nc


_NC = None


def _get_nc() -> bass.Bass:
    global _NC
    if _NC is None:
        _NC = build_nc()
    return _NC


def kernel(**inputs: np.ndarray) -> np.ndarray:
    x = np.ascontiguousarray(inputs["inputs"], dtype=np.float32)
    assert x.shape == (B, T, F), x.shape
    nc = _get_nc()
    in_maps = [{"inp": x[c]} for c in range(B)]
    res = run_bass_kernel_spmd(nc, in_maps, core_ids=list(range(B)))
    return np.stack(
        [np.asarray(res.results[c]["out"]).astype(np.float32) for c in range(B)],
        axis=0,
    )


# revision 22
# speedup vs baseline: 1.4876x; 1.4876x over previous
"""Trainium2 Bass kernel for CLSProcess: diagonal linear recurrence
state_t = y_t * state_{t-1} + x_t * z_t over [B=8, T=4096, units=1024].

Sharding: batch across the 8 cores (one batch element per core).

v4 design:
  - the ONLY input DMAs are full-row gpsimd SWDGE cast-DMAs (f32->bf16
    in flight), one per 4-block group with a "(a b) c -> b a c"
    rearrange (4104-byte descriptors).  4-byte column gathers are
    catastrophic (the DMA queue burns ~7-16ns per tiny descriptor
    serially, ~55us for a [4096,1] column) and are completely avoided.
  - x / y0 columns per block come from one strided [128,4,2] engine
    copy per group (bf16 -> f32); the y ROW needed by the M-scan comes
    from a per-block PE transpose ([128,1] -> [1,128] bf16) + one Act
    row copy into a persistent [1, 32, 128] f32 row buffer whose
    block-start positions stay 0 (scan reset).
  - all matmuls bf16 (f32r lowers to 4-pass fp32 on this toolchain).
    lhsT = msc[s,t] = x_s*M[t,s] via a DVE tensor_scalar; M built by a
    DVE tensor_tensor_scan over the identity, 4 blocks per scan.
  - carry: po[t,:] += p_t*prev[127,:] as a K=128 rank-1 bf16 matmul.
    sel tiles are 10 rotating statics: rows 0:64 zeroed once at
    prologue, rows 64:128 rewritten per block (gpsimd broadcast of
    p_t + masked Act activation) -- halves the broadcast cost.
  - software pipeline: scans 2 groups ahead, per-block prep 8 blocks
    ahead, carries 2 blocks behind mains (PE never waits on drains),
    output bf16 (host upconverts), 2 blocks per output DMA on the SP
    queue.
"""

import numpy as np

import concourse.bacc as bacc
import concourse.bass as bass
import concourse.mybir as mybir
import concourse.tile as tile
from concourse.bass_utils import run_bass_kernel_spmd

B = 8
T = 4096
F = 1026
U = 1024
L = 128
G = 4  # blocks per group (one scan / one cast-DMA per group)
OB = 2  # blocks per output DMA
NSEL = 12
f32 = mybir.dt.float32
f32r = mybir.dt.float32r
bf16 = mybir.dt.bfloat16
Copy = mybir.ActivationFunctionType.Copy


def build_nc(t_total: int = T) -> bass.Bass:
    nb = t_total // L
    ng = nb // G
    nc = bacc.Bacc()
    inp = nc.dram_tensor("inp", [t_total, F], f32, kind="ExternalInput")
    out = nc.dram_tensor("out", [t_total, U], bf16, kind="ExternalOutput")
    ident4_d = nc.inline_tensor(
        np.tile(np.eye(L, dtype=np.float32), (1, G)), name="ident4"
    )
    e127c_np = np.zeros((L, 1), dtype=np.float32)
    e127c_np[L - 1, 0] = 1.0
    e127c_d = nc.inline_tensor(e127c_np, name="e127c")

    with tile.TileContext(nc) as tc:
        with (
            tc.tile_pool(name="const", bufs=1) as constp,
            tc.tile_pool(name="yrow", bufs=1) as yrowp,
            tc.tile_pool(name="zpool", bufs=8) as zpool,
            tc.tile_pool(name="xyfpool", bufs=5) as xyfpool,
            tc.tile_pool(name="mpool", bufs=3) as mpool,
            tc.tile_pool(name="mscpool", bufs=14) as mscpool,
            tc.tile_pool(name="rowpool", bufs=14) as rowpool,
            tc.tile_pool(name="bcpool", bufs=4) as bcpool,
            tc.tile_pool(name="pbpool", bufs=14) as pbpool,
            tc.tile_pool(name="selpool", bufs=14) as selpool,
            tc.tile_pool(name="otbpool", bufs=3) as otbpool,
            tc.tile_pool(name="ps_out", bufs=6, space="PSUM") as ps_out_pool,
            tc.tile_pool(name="ps_small", bufs=2, space="PSUM") as ps_small_pool,
        ):
            ident4 = constp.tile([L, G * L], f32, tag="ident4")
            nc.sync.dma_start(ident4[:], ident4_d[:, :])
            e127c = constp.tile([L, 1], f32, tag="e127c")
            nc.sync.dma_start(e127c[:], e127c_d[:, :])
            # bf16 identity for the bf16 y-column transposes (gpsimd
            # cast-DMA: engines cannot mix 16/32-bit matmul operands)
            identb = constp.tile([L, L], bf16, tag="identb")
            nc.gpsimd.dma_start(identb[:], ident4_d[:, 0:L])

            # persistent y rows, [1, block, step]; block-start positions
            # stay 0 so the M-scan resets at each block boundary
            yrow3 = yrowp.tile([1, nb, L], f32, tag="yrow3")
            nc.vector.memset(yrow3[0:1, :, 0:1], 0.0)

            zts = {}
            xyfs = {}
            ybcs = {}
            mt4s = {}
            mscs = {}
            sels = {}
            pos = {}
            otbs = {}

            def dispatch_z(g: int):
                r0 = g * G * L
                zt = zpool.tile([L, G * F], bf16, tag="zt")
                nc.gpsimd.dma_start(
                    zt[:],
                    inp[r0 : r0 + G * L, 0:F].rearrange("(a b) c -> b a c", a=G),
                )
                zts[g] = zt

            def prep_y(g: int):
                # x/y0 sideband columns (f32) + y rows + broadcast for group g
                zt = zts[g]
                z3 = zt[:].rearrange("b (a c) -> b a c", a=G)
                xyf = xyfpool.tile([L, G * 2], f32, tag="xyf")
                nc.vector.tensor_copy(xyf[:], z3[:, :, 0:2])
                xyfs[g] = xyf
                for j in range(G):
                    k = g * G + j
                    psm = ps_small_pool.tile([1, L], bf16, tag="psm")
                    nc.tensor.transpose(psm[0:1, :], zt[:, j * F + 1 : j * F + 2], identb[:])
                    nc.scalar.copy(yrow3[0:1, k : k + 1, 1:L], psm[0:1, 1:L])
                ybc = bcpool.tile([L, G * L], f32, tag="ybc")
                nc.gpsimd.partition_broadcast(
                    ybc[:], yrow3[0:1, G * g : G * (g + 1), :]
                )
                ybcs[g] = ybc

            def scan_group(g: int):
                mt4 = mpool.tile([L, G * L], f32r, tag="mt4")
                nc.vector.tensor_tensor_scan(
                    mt4[:],
                    ybcs.pop(g)[:],
                    ident4[:],
                    0.0,
                    mybir.AluOpType.mult,
                    mybir.AluOpType.add,
                )
                mt4s[g] = mt4

            def prep_block(k: int):
                # weights + carry selector for block k (runs well ahead of PE)
                g, j = divmod(k, G)
                mt4 = mt4s[g]
                mtk = mt4[:, L * j : L * j + L]
                xyf = xyfs[g]
                # bf16 weights: msc[s, t] = x_s * M[t, s]
                msc = mscpool.tile([L, L], bf16, tag="msc")
                nc.vector.tensor_scalar_mul(msc[:], mtk, xyf[:, 2 * j : 2 * j + 1])
                mscs[k] = msc
                if k > 0:
                    # p_t = prod_{r=block_start..t} y_r = y_0 * mt[0, t]
                    y0 = xyf[0:1, 2 * j + 1 : 2 * j + 2]
                    prow = rowpool.tile([1, L], f32, tag="prow")
                    nc.vector.tensor_scalar_mul(prow[:], mtk[0:1, :], y0)
                    # sel[s, t] = I[s==127] * p_t
                    pb = pbpool.tile([L, L], f32, tag="pb")
                    nc.gpsimd.partition_broadcast(pb[:], prow[0:1, :])
                    sel = selpool.tile([L, L], bf16, tag="sel")
                    nc.scalar.activation(sel[:], pb[:], Copy, scale=e127c[:])
                    sels[k] = sel

            def mains(k: int):
                g, j = divmod(k, G)
                msc = mscs.pop(k)
                zt = zts[g]
                halves = []
                for jj in (0, 512):
                    po = ps_out_pool.tile([L, 512], f32, tag="po")
                    nc.tensor.matmul(
                        po[:],
                        msc[:],
                        zt[:, j * F + 2 + jj : j * F + 2 + jj + 512],
                        start=True,
                        stop=(k == 0),
                    )
                    halves.append(po)
                pos[k] = halves

            def back(k: int):
                # carry accumulation + drain + output for block k; the two
                # 512-column halves form independent chains (drain half 0 on
                # DVE feeds carry half 0, half 1 on Act feeds carry half 1)
                halves = pos.pop(k)
                if k > 0:
                    # po[t, :] += p_t * prev[127, :]
                    sel = sels.pop(k)
                    pt, pc = otbs[k - 1]
                    for hh, jj in enumerate((0, 512)):
                        nc.tensor.matmul(
                            halves[hh][:],
                            sel[:],
                            pt[:, pc + jj : pc + jj + 512],
                            start=False,
                            stop=True,
                        )
                    otbs.pop(k - 1, None)
                h = k % OB
                if h == 0:
                    otb = otbpool.tile([L, OB * U], bf16, tag="otb")
                    otbs["cur"] = otb
                otb = otbs["cur"]
                c0 = h * U
                nc.vector.tensor_copy(otb[:, c0 : c0 + 512], halves[0][:])
                nc.scalar.copy(otb[:, c0 + 512 : c0 + U], halves[1][:])
                otbs[k] = (otb, c0)
                if h == OB - 1:
                    r0b = (k - OB + 1) * L
                    nc.sync.dma_start(
                        out[r0b : r0b + OB * L, :].rearrange(
                            "(a b) c -> b a c", a=OB
                        ),
                        otb[:],
                    )

            # prologue: all z cast-DMAs dispatched back-to-back on the
            # SWDGE queue; y/scan/weights prep pipelined ahead
            for g in range(ng):
                dispatch_z(g)
            prep_y(0)
            prep_y(1)
            prep_y(2)
            scan_group(0)
            scan_group(1)
            PREP_AHEAD = 8
            for kk in range(PREP_AHEAD):
                prep_block(kk)
            for k in range(nb):
                g = k // G
                if k % G == 0:
                    if g + 3 <= ng - 1:
                        prep_y(g + 3)
                    if g + 2 <= ng - 1:
                        scan_group(g + 2)
                if k + PREP_AHEAD < nb:
                    prep_block(k + PREP_AHEAD)
                mains(k)
                if k >= 2:
                    back(k - 2)
            back(nb - 2)
            back(nb - 1)
    nc.finalize()
    return nc


_NC = None


def _get_nc() -> bass.Bass:
    global _NC
    if _NC is None:
        _NC = build_nc()
    return _NC


def kernel(**inputs: np.ndarray) -> np.ndarray:
    x = np.ascontiguousarray(inputs["inputs"], dtype=np.float32)
    assert x.shape == (B, T, F), x.shape
    nc = _get_nc()
    in_maps = [{"inp": x[c]} for c in range(B)]
    res = run_bass_kernel_spmd(nc, in_maps, core_ids=list(range(B)))
    return np.stack(
        [np.asarray(res.results[c]["out"]).astype(np.float32) for c in range(B)],
        axis=0,
    )
